# revision 24
# baseline (speedup 1.0000x reference)
"""Grouped multivariate kernel-CRPS loss on 8 TRN2 NeuronCores.

Sharding: latlon (20480) split across 8 cores (2560 each). Host pre-folds the
feature weights into the data during the fp32->bf16 cast and relays out each
(b,t) tile as one contiguous [128 partitions x 12*640] block (slot 0 = target,
slots 1-8 = ensembles, slots 9-11 = wrap copies of ensembles 0-2), so every
DMA is one wide sequential transfer instead of ~1k small strided descriptors.
On device, the 36 unique pair diffs per point are 4 wide DVE subtracts per
(b,t) tile via the circular-distance slot layout; |w|^1.5 = exp(0.75 ln(w^2))
with the square on DVE and Ln/Exp on the Activation engine (both share one
activation table, so no table reloads); the grouped K-sum is one native
tensor_reduce per half-tile. Final S^(2/3) applies the 1/8 (error) and -1/56
(spread) weights via the Exp bias; node weighting and the final reduction run
once per kernel on [128, 2880] accumulators.
"""
import sys
sys.path.insert(0, '/opt/trn_rl_repo')
import math
import numpy as np
import ml_dtypes

import concourse.bacc as bacc
import concourse.mybir as mybir
from concourse.tile import TileContext
from concourse.bass_utils import run_bass_kernel_spmd
import bass_rust

F32 = mybir.dt.float32
BF16 = mybir.dt.bfloat16
Alu = mybir.AluOpType
Act = mybir.ActivationFunctionType

B, E, T, LATLON, K = 2, 8, 2, 20480, 32
NCORES = 8
SHARD = LATLON // NCORES
LPP = SHARD // 128
SL = LPP * K
NSLOT = 12
INW = NSLOT * SL
GRP = 36 * LPP
NT = B * T
WA_ = 16 * SL
WB_ = 20 * SL

_CACHE = {}


def _ap(base, pairs, off):
    c = base.copy()
    c.ap = bass_rust.VecI64Pair(pairs)
    c.offset = off
    return c


def build(reps=1, sq_engine='dve'):
    key = (reps, sq_engine)
    if key in _CACHE:
        return _CACHE[key]
    nc = bacc.Bacc()
    x = nc.dram_tensor("x", [NT, 128, INW], BF16, kind="ExternalInput")
    nwc = nc.dram_tensor("nwc", [128, LPP], F32, kind="ExternalInput")
    out = nc.dram_tensor("out", [128, 1], F32, kind="ExternalOutput")

    with TileContext(nc) as tc:
        with tc.tile_pool(name="const", bufs=1) as cp, \
             tc.tile_pool(name="inp", bufs=2) as ip, \
             tc.tile_pool(name="wk", bufs=1) as wkp, \
             tc.tile_pool(name="sk", bufs=1) as skp, \
             tc.tile_pool(name="acc", bufs=1) as ap_:
            NWT = cp.tile([128, LPP], F32, tag="NWT")
            nc.sync.dma_start(out=NWT[:], in_=nwc[:])
            BIASE = cp.tile([128, 1], F32, tag="BIASE")
            nc.vector.memset(BIASE[:], math.log(1.0 / 8.0))
            BIASD = cp.tile([128, 1], F32, tag="BIASD")
            nc.vector.memset(BIASD[:], math.log(1.0 / 56.0))
            EPSB = cp.tile([128, 1], F32, tag="EPSB")
            nc.vector.memset(EPSB[:], 1e-30)
            SACC = ap_.tile([128, NT * GRP], F32, tag="SACC")

            for rep in range(reps):
                for bt in range(NT):
                    IN = ip.tile([128, INW], BF16, tag="IN")
                    nc.sync.dma_start(out=IN[:], in_=_ap(
                        x[:], [(INW, 128), (1, INW)], bt * 128 * INW))

                    WAt = wkp.tile([128, WA_], BF16, tag="WA")
                    SAt = skp.tile([128, WA_], BF16, tag="SA")
                    nc.vector.tensor_tensor(
                        _ap(WAt[:], [(WA_, 128), (SL, 8), (1, SL)], 0),
                        _ap(IN[:], [(INW, 128), (0, 8), (1, SL)], 0),
                        _ap(IN[:], [(INW, 128), (SL, 8), (1, SL)], SL),
                        Alu.subtract)
                    nc.vector.tensor_tensor(
                        _ap(WAt[:], [(WA_, 128), (SL, 8), (1, SL)], 8 * SL),
                        _ap(IN[:], [(INW, 128), (SL, 8), (1, SL)], SL),
                        _ap(IN[:], [(INW, 128), (SL, 8), (1, SL)], 2 * SL),
                        Alu.subtract)
                    nc.vector.tensor_tensor(SAt[:], WAt[:], WAt[:], Alu.mult)
                    nc.scalar.activation(WAt[:], SAt[:], Act.Ln, bias=EPSB[:])
                    nc.scalar.activation(SAt[:], WAt[:], Act.Exp, scale=0.75)
                    nc.vector.tensor_reduce(
                        SACC[:, bt * GRP:bt * GRP + 16 * LPP],
                        SAt[:].rearrange("p (g k) -> p g k", k=K),
                        axis=mybir.AxisListType.X, op=Alu.add)

                    WBt = wkp.tile([128, WB_], BF16, tag="WB")
                    SBt = skp.tile([128, WB_], BF16, tag="SB")
                    nc.vector.tensor_tensor(
                        _ap(WBt[:], [(WB_, 128), (8 * SL, 2), (SL, 8), (1, SL)], 0),
                        _ap(IN[:], [(INW, 128), (0, 2), (SL, 8), (1, SL)], SL),
                        _ap(IN[:], [(INW, 128), (SL, 2), (SL, 8), (1, SL)], 3 * SL),
                        Alu.subtract)
                    nc.vector.tensor_tensor(
                        _ap(WBt[:], [(WB_, 128), (SL, 4), (1, SL)], 16 * SL),
                        _ap(IN[:], [(INW, 128), (SL, 4), (1, SL)], SL),
                        _ap(IN[:], [(INW, 128), (SL, 4), (1, SL)], 5 * SL),
                        Alu.subtract)
                    nc.vector.tensor_tensor(SBt[:], WBt[:], WBt[:], Alu.mult)
                    nc.scalar.activation(WBt[:], SBt[:], Act.Ln, bias=EPSB[:])
                    nc.scalar.activation(SBt[:], WBt[:], Act.Exp, scale=0.75)
                    nc.vector.tensor_reduce(
                        SACC[:, bt * GRP + 16 * LPP:(bt + 1) * GRP],
                        SBt[:].rearrange("p (g k) -> p g k", k=K),
                        axis=mybir.AxisListType.X, op=Alu.add)

            LNS = ap_.tile([128, NT * GRP], F32, tag="LNS")
            nc.scalar.activation(LNS[:], SACC[:], Act.Ln)
            NPW = ap_.tile([128, NT * GRP], F32, tag="NPW")
            t3 = NPW[:].rearrange("p (t g) -> p t g", g=GRP)
            l3 = LNS[:].rearrange("p (t g) -> p t g", g=GRP)
            EC = 8 * LPP
            nc.scalar.activation(t3[:, :, 0:EC], l3[:, :, 0:EC],
                                 Act.Exp, scale=2.0 / 3.0, bias=BIASE[:])
            nc.scalar.activation(t3[:, :, EC:GRP], l3[:, :, EC:GRP],
                                 Act.Exp, scale=2.0 / 3.0, bias=BIASD[:])
            nc.vector.tensor_scalar(
                t3[:, :, EC:GRP], t3[:, :, EC:GRP], -1.0, None, Alu.mult)
            KW = ap_.tile([128, NT * GRP], F32, tag="KW")
            nc.vector.tensor_tensor(
                KW[:].rearrange("p (t g l) -> p t g l", t=NT, l=LPP),
                NPW[:].rearrange("p (t g l) -> p t g l", t=NT, l=LPP),
                _ap(NWT[:], [(LPP, 128), (0, NT), (0, 36), (1, LPP)], 0),
                Alu.mult)
            GR = ap_.tile([128, 1], F32, tag="GR")
            nc.vector.tensor_reduce(GR[:], KW[:], axis=mybir.AxisListType.X, op=Alu.add)
            nc.sync.dma_start(out=out[:, :], in_=GR[:])
    nc.finalize()
    _CACHE[key] = nc
    return nc


def _prep(preds, target, node_weights, feature_weights):
    fwn = (feature_weights.astype(np.float32) / feature_weights.size)
    pr = (preds.astype(np.float32) * fwn).astype(ml_dtypes.bfloat16)
    tg = (target.astype(np.float32) * fwn).astype(ml_dtypes.bfloat16)
    pr = pr.reshape(B, E, T, NCORES, 128, LPP, K).transpose(3, 0, 2, 4, 1, 5, 6)
    tg = tg.reshape(B, 1, T, NCORES, 128, LPP, K).transpose(3, 0, 2, 4, 1, 5, 6)
    X = np.empty((NCORES, NT, 128, NSLOT, LPP, K), dtype=ml_dtypes.bfloat16)
    Xv = X.reshape(NCORES, B, T, 128, NSLOT, LPP, K)
    Xv[:, :, :, :, 0] = tg[:, :, :, :, 0]
    Xv[:, :, :, :, 1:9] = pr
    Xv[:, :, :, :, 9:12] = pr[:, :, :, :, 0:3]
    nwf = node_weights.astype(np.float32)
    nwc = nwf.reshape(NCORES, 128, LPP)
    return X.reshape(NCORES, NT, 128, INW), nwc, nwf


def kernel(preds, target, node_weights, feature_weights, _reps=1, **kw):
    nc = build(_reps)
    X, nwc, nwf = _prep(np.asarray(preds), np.asarray(target),
                        np.asarray(node_weights), np.asarray(feature_weights))
    in_maps = [{"x": np.ascontiguousarray(X[c]),
                "nwc": np.ascontiguousarray(nwc[c])} for c in range(NCORES)]
    res = run_bass_kernel_spmd(nc, in_maps, core_ids=list(range(NCORES)))
    total = sum(float(r["out"].sum()) for r in res.results)
    total = total / float(nwf.sum()) / B
    return np.float32(total)


# revision 25
# speedup vs baseline: 3.4755x; 3.4755x over previous
"""Grouped multivariate kernel-CRPS loss on 8 TRN2 NeuronCores.

Sharding: latlon (20480) split across 8 cores (2560 each). Host pre-folds the
feature weights into the data during the fp32->bf16 cast and relays out each
(b,t) tile as one contiguous [128 partitions x 12*640] block (slot 0 = target,
slots 1-8 = ensembles, slots 9-11 = wrap copies of ensembles 0-2), so every
DMA is one wide sequential transfer instead of ~1k small strided descriptors.
Per (b,t) tile, just 9 wide instructions: 4 DVE subtracts produce all 36
unique pair diffs via the circular-distance slot layout into one full-width
[128 x 23040] tile (double-buffered so tile n+1's subtracts overlap tile n's
activations), then |w|^1.5 = exp(0.75 ln(w^2)) as one DVE square + one Ln +
one Exp on the Activation engine (shared act table, no reloads), and one
grouped tensor_reduce for the K-sums. Final S^(2/3) applies the 1/8 (error)
and -1/56 (spread) weights via the Exp bias, written in-place over the
accumulator; node weighting and the final reduction run once per kernel.
"""
import sys
sys.path.insert(0, '/opt/trn_rl_repo')
import math
import numpy as np
import ml_dtypes

import concourse.bacc as bacc
import concourse.mybir as mybir
from concourse.tile import TileContext
from concourse.bass_utils import run_bass_kernel_spmd
import bass_rust

F32 = mybir.dt.float32
BF16 = mybir.dt.bfloat16
Alu = mybir.AluOpType
Act = mybir.ActivationFunctionType

B, E, T, LATLON, K = 2, 8, 2, 20480, 32
NCORES = 8
SHARD = LATLON // NCORES
LPP = SHARD // 128
SL = LPP * K
NSLOT = 12
INW = NSLOT * SL
GRP = 36 * LPP
NT = B * T
WA_ = 16 * SL
WB_ = 20 * SL

_CACHE = {}


def _ap(base, pairs, off):
    c = base.copy()
    c.ap = bass_rust.VecI64Pair(pairs)
    c.offset = off
    return c


def build(reps=1, sq_engine='dve'):
    key = (reps, sq_engine)
    if key in _CACHE:
        return _CACHE[key]
    nc = bacc.Bacc()
    x = nc.dram_tensor("x", [NT, 128, INW], BF16, kind="ExternalInput")
    nwc = nc.dram_tensor("nwc", [128, LPP], F32, kind="ExternalInput")
    out = nc.dram_tensor("out", [128, 1], F32, kind="ExternalOutput")

    with TileContext(nc) as tc:
        with tc.tile_pool(name="const", bufs=1) as cp, \
             tc.tile_pool(name="inp", bufs=2) as ip, \
             tc.tile_pool(name="wk", bufs=2) as wkp, \
             tc.tile_pool(name="sk", bufs=1) as skp, \
             tc.tile_pool(name="acc", bufs=1) as ap_:
            NWT = cp.tile([128, LPP], F32, tag="NWT")
            nc.sync.dma_start(out=NWT[:], in_=nwc[:])
            BIASE = cp.tile([128, 1], F32, tag="BIASE")
            nc.vector.memset(BIASE[:], math.log(1.0 / 8.0))
            BIASD = cp.tile([128, 1], F32, tag="BIASD")
            nc.vector.memset(BIASD[:], math.log(1.0 / 56.0))
            EPSB = cp.tile([128, 1], F32, tag="EPSB")
            nc.vector.memset(EPSB[:], 1e-30)
            SACC = ap_.tile([128, NT * GRP], F32, tag="SACC")

            for rep in range(reps):
                for bt in range(NT):
                    IN = ip.tile([128, INW], BF16, tag="IN")
                    nc.sync.dma_start(out=IN[:], in_=_ap(
                        x[:], [(INW, 128), (1, INW)], bt * 128 * INW))

                    WW_ = 36 * SL
                    Wt = wkp.tile([128, WW_], BF16, tag="W")
                    St = skp.tile([128, WW_], BF16, tag="S")
                    nc.vector.tensor_tensor(
                        _ap(Wt[:], [(WW_, 128), (SL, 8), (1, SL)], 0),
                        _ap(IN[:], [(INW, 128), (0, 8), (1, SL)], 0),
                        _ap(IN[:], [(INW, 128), (SL, 8), (1, SL)], SL),
                        Alu.subtract)
                    nc.vector.tensor_tensor(
                        _ap(Wt[:], [(WW_, 128), (SL, 8), (1, SL)], 8 * SL),
                        _ap(IN[:], [(INW, 128), (SL, 8), (1, SL)], SL),
                        _ap(IN[:], [(INW, 128), (SL, 8), (1, SL)], 2 * SL),
                        Alu.subtract)
                    nc.vector.tensor_tensor(
                        _ap(Wt[:], [(WW_, 128), (8 * SL, 2), (SL, 8), (1, SL)], 16 * SL),
                        _ap(IN[:], [(INW, 128), (0, 2), (SL, 8), (1, SL)], SL),
                        _ap(IN[:], [(INW, 128), (SL, 2), (SL, 8), (1, SL)], 3 * SL),
                        Alu.subtract)
                    nc.vector.tensor_tensor(
                        _ap(Wt[:], [(WW_, 128), (SL, 4), (1, SL)], 32 * SL),
                        _ap(IN[:], [(INW, 128), (SL, 4), (1, SL)], SL),
                        _ap(IN[:], [(INW, 128), (SL, 4), (1, SL)], 5 * SL),
                        Alu.subtract)
                    nc.vector.tensor_tensor(St[:], Wt[:], Wt[:], Alu.mult)
                    nc.scalar.activation(Wt[:], St[:], Act.Ln, bias=EPSB[:])
                    nc.scalar.activation(St[:], Wt[:], Act.Exp, scale=0.75)
                    nc.vector.tensor_reduce(
                        SACC[:, bt * GRP:(bt + 1) * GRP],
                        St[:].rearrange("p (g k) -> p g k", k=K),
                        axis=mybir.AxisListType.X, op=Alu.add)

            LNS = ap_.tile([128, NT * GRP], F32, tag="LNS")
            nc.scalar.activation(LNS[:], SACC[:], Act.Ln)
            t3 = SACC[:].rearrange("p (t g) -> p t g", g=GRP)
            l3 = LNS[:].rearrange("p (t g) -> p t g", g=GRP)
            EC = 8 * LPP
            nc.scalar.activation(t3[:, :, 0:EC], l3[:, :, 0:EC],
                                 Act.Exp, scale=2.0 / 3.0, bias=BIASE[:])
            nc.scalar.activation(t3[:, :, EC:GRP], l3[:, :, EC:GRP],
                                 Act.Exp, scale=2.0 / 3.0, bias=BIASD[:])
            nc.vector.tensor_scalar(
                t3[:, :, EC:GRP], t3[:, :, EC:GRP], -1.0, None, Alu.mult)
            KW = LNS
            nc.vector.tensor_tensor(
                KW[:].rearrange("p (t g l) -> p t g l", t=NT, l=LPP),
                SACC[:].rearrange("p (t g l) -> p t g l", t=NT, l=LPP),
                _ap(NWT[:], [(LPP, 128), (0, NT), (0, 36), (1, LPP)], 0),
                Alu.mult)
            GR = ap_.tile([128, 1], F32, tag="GR")
            nc.vector.tensor_reduce(GR[:], KW[:], axis=mybir.AxisListType.X, op=Alu.add)
            nc.sync.dma_start(out=out[:, :], in_=GR[:])
    nc.finalize()
    _CACHE[key] = nc
    return nc


def _prep(preds, target, node_weights, feature_weights):
    fwn = (feature_weights.astype(np.float32) / feature_weights.size)
    pr = (preds.astype(np.float32) * fwn).astype(ml_dtypes.bfloat16)
    tg = (target.astype(np.float32) * fwn).astype(ml_dtypes.bfloat16)
    pr = pr.reshape(B, E, T, NCORES, 128, LPP, K).transpose(3, 0, 2, 4, 1, 5, 6)
    tg = tg.reshape(B, 1, T, NCORES, 128, LPP, K).transpose(3, 0, 2, 4, 1, 5, 6)
    X = np.empty((NCORES, NT, 128, NSLOT, LPP, K), dtype=ml_dtypes.bfloat16)
    Xv = X.reshape(NCORES, B, T, 128, NSLOT, LPP, K)
    Xv[:, :, :, :, 0] = tg[:, :, :, :, 0]
    Xv[:, :, :, :, 1:9] = pr
    Xv[:, :, :, :, 9:12] = pr[:, :, :, :, 0:3]
    nwf = node_weights.astype(np.float32)
    nwc = nwf.reshape(NCORES, 128, LPP)
    return X.reshape(NCORES, NT, 128, INW), nwc, nwf


def kernel(preds, target, node_weights, feature_weights, _reps=1, **kw):
    nc = build(_reps)
    X, nwc, nwf = _prep(np.asarray(preds), np.asarray(target),
                        np.asarray(node_weights), np.asarray(feature_weights))
    in_maps = [{"x": np.ascontiguousarray(X[c]),
                "nwc": np.ascontiguousarray(nwc[c])} for c in range(NCORES)]
    res = run_bass_kernel_spmd(nc, in_maps, core_ids=list(range(NCORES)))
    total = sum(float(r["out"].sum()) for r in res.results)
    total = total / float(nwf.sum()) / B
    return np.float32(total)
